# revision 18
# baseline (speedup 1.0000x reference)
"""Multi-head attention (B=2, S=4096, H=8, d_head=16) on 8 Trainium2 cores.

Sharding: core -> (batch b = core//4, query quarter of 1024). Each core
computes all 8 heads for its 1024 queries. K/V for the core's batch are
fully resident in SBUF (mask-compacted).

Math notes:
  - seq_mask==0 keys get -1e30 on their logits -> weight 0. We compact K/V
    on host to the valid keys (~50%), padded to a multiple of 128; pad keys
    carry -1e30 in an augmented contraction channel (d 16->17, Q channel
    16 == 1.0) so exp() kills them on device.
  - The scalar bias `b` shifts every logit equally; softmax is
    shift-invariant so it cancels exactly and is not sent to the device.
  - Softmax max-subtraction is skipped: logits ~ N(0,1); exp cannot
    overflow fp32 and the reference max-subtraction cancels identically.

Device dataflow per (q-tile of 512, head pair, key-chunk triple):
  QK^T:  lt[key 128, 3*512] = kt[17,128].T @ qt[17,512]   (PE, f32r,
         4-way row-tiled via K^T/Q^T replicas at partitions 0/32/64/96)
  exp:   e = Exp(lt)  -> bf16                             (ACT, FD=1536)
  PV:    acc[0:33 | 64:97, 512] += va[128,33].T @ e       (PE, bf16,
         2-way col-tiled: head pair packed in one PSUM bank)
         (va col 32 == 1.0 -> acc rows 32/96 = softmax denominators)
  PV is emitted one key-triple behind QK/exp so the PE never blocks the
  ACT pipeline (PE queues are strict FIFO).
  out:   evac acc -> SBUF, reciprocal rows, DMA-replicate broadcast,
         multiply, DMA out.
"""

import sys

import numpy as np

if "/opt/trn_rl_repo" not in sys.path:
    sys.path.insert(0, "/opt/trn_rl_repo")

import ml_dtypes

BF16 = ml_dtypes.bfloat16

UNITS = 128
H = 8
DH = 16
B = 2
S = 4096
QPC = 1024  # queries per core (B*S / 8 cores)
QT = 512    # q tile (fp32 moving-operand max on PE)
VW = 33     # V_aug width: V at 0..15, ones at 32 (APs need 32-aligned bases)
KT3 = 3     # key chunks per lt supertile (3 PSUM banks, ACT FD=1536)
NEG = -1.0e30

TRACE = False
TMPDIR = None
LAST = None

_compiled = {}


def _build(NC):
    import concourse.bass as bass
    import concourse.tile as tile
    from concourse import bacc, mybir

    f32 = mybir.dt.float32
    f32r = mybir.dt.float32r
    bf16 = mybir.dt.bfloat16
    NK = NC * 128

    nc = bacc.Bacc()
    kt = nc.dram_tensor("kt", [17, H, NK], f32r, kind="ExternalInput")
    qt = nc.dram_tensor("qt", [17, H, QPC], f32r, kind="ExternalInput")
    va = nc.dram_tensor("va", [NC, 128, H * VW], bf16, kind="ExternalInput")
    out = nc.dram_tensor("out", [H, QPC // QT, DH, QT], f32, kind="ExternalOutput")

    trips = [list(range(t, min(t + KT3, NC))) for t in range(0, NC, KT3)]

    with tile.TileContext(nc) as tc:
        with (
            tc.tile_pool(name="const", bufs=1) as cpool,
            tc.tile_pool(name="lt", bufs=2, space="PSUM") as lt_pool,
            tc.tile_pool(name="acc", bufs=2, space="PSUM") as acc_pool,
            tc.tile_pool(name="exp", bufs=3) as exp_pool,
            tc.tile_pool(name="div", bufs=6) as div_pool,
            tc.tile_pool(name="res", bufs=3) as res_pool,
        ):
            # K^T / Q^T replicated at partition bases 0/32/64/96 for 4-way
            # row-tiled QK matmuls.
            kt_sb = cpool.tile([128, H, NK], f32r)
            qt_sb = cpool.tile([128, H, QPC], f32r)
            for r in range(4):
                nc.sync.dma_start(out=kt_sb[32 * r : 32 * r + 17, :, :], in_=kt[:, :, :])
                nc.sync.dma_start(out=qt_sb[32 * r : 32 * r + 17, :, :], in_=qt[:, :, :])
            va_sb = cpool.tile([128, NC, H * VW], bf16)
            nc.sync.dma_start(out=va_sb, in_=va[:, :, :].rearrange("c p f -> p c f"))

            rg = 0  # row-group rotation for QK weight placement
            for qi in range(QPC // QT):
                for hg in range(H // 2):
                    h0, h1 = 2 * hg, 2 * hg + 1
                    acc = acc_pool.tile([128, QT], f32, name=f"acc_{qi}_{hg}", tag="acc")
                    pend = None  # (lts, ets, kcs) waiting for PV emission
                    for kcs in trips:
                        w = len(kcs) * QT
                        lts = []
                        ets = []
                        for hi, h in enumerate((h0, h1)):
                            lt_t = lt_pool.tile(
                                [128, KT3 * QT], f32, name=f"lt_{hi}", tag="lt"
                            )
                            for j, kc in enumerate(kcs):
                                r = rg % 4
                                rg += 1
                                nc.tensor.matmul(
                                    lt_t[:, j * QT : (j + 1) * QT],
                                    lhsT=kt_sb[
                                        32 * r : 32 * r + 17, h, kc * 128 : (kc + 1) * 128
                                    ],
                                    rhs=qt_sb[
                                        32 * r : 32 * r + 17, h, qi * QT : (qi + 1) * QT
                                    ],
                                    start=True,
                                    stop=True,
                                    tile_position=(32 * r, 0),
                                )
                            lts.append(lt_t)
                        for hi, h in enumerate((h0, h1)):
                            e_t = exp_pool.tile(
                                [128, KT3 * QT], bf16, name=f"e_{hi}", tag="e"
                            )
                            nc.scalar.activation(
                                e_t[:, :w], lts[hi][:, :w],
                                mybir.ActivationFunctionType.Exp,
                            )
                            ets.append(e_t)
                        if pend is not None:
                            _emit_pv(nc, acc, va_sb, pend, h0, h1, NC)
                        pend = (lts, ets, kcs)
                    _emit_pv(nc, acc, va_sb, pend, h0, h1, NC)

                    # fused division for the head pair
                    ev = div_pool.tile([128, QT], f32, name="ev", tag="ev")
                    nc.vector.tensor_copy(ev, acc[:, :])
                    rec = div_pool.tile([128, QT], f32, name="rec", tag="rec")
                    nc.vector.reciprocal(rec, ev)
                    rb = div_pool.tile([128, QT], f32, name="rb", tag="rb")
                    for hi, h in enumerate((h0, h1)):
                        src = rec[64 * hi + 32 : 64 * hi + 33, :]
                        bsrc = bass.AP(
                            tensor=src.tensor,
                            offset=src.offset,
                            ap=[src.ap[0], [0, DH]] + src.ap[1:],
                        )
                        nc.sync.dma_start(out=rb[64 * hi : 64 * hi + DH, :], in_=bsrc)
                    o_t = res_pool.tile([128, QT], f32, name="o_t", tag="o")
                    nc.vector.tensor_mul(o_t, ev, rb)
                    for hi, h in enumerate((h0, h1)):
                        nc.sync.dma_start(
                            out=out[h, qi], in_=o_t[64 * hi : 64 * hi + DH, :]
                        )
    nc.compile()
    return nc


def _emit_pv(nc, acc, va_sb, pend, h0, h1, NC):
    lts, ets, kcs = pend
    for hi, h in enumerate((h0, h1)):
        for j, kc in enumerate(kcs):
            nc.tensor.matmul(
                acc[64 * hi : 64 * hi + VW, :],
                lhsT=va_sb[:, kc, h * VW : (h + 1) * VW],
                rhs=ets[hi][:, j * QT : (j + 1) * QT],
                start=(kc == 0),
                stop=(kc == NC - 1),
                tile_position=(0, 64 * hi),
            )


def _get_compiled(NC):
    if NC not in _compiled:
        _compiled[NC] = _build(NC)
    return _compiled[NC]


def kernel(memory, query, seq_mask, b):
    global LAST
    memory = np.asarray(memory, dtype=np.float32)
    query = np.asarray(query, dtype=np.float32)
    seq_mask = np.asarray(seq_mask)

    idx = [np.flatnonzero(seq_mask[bb] != 0) for bb in range(B)]
    nv = [len(i) for i in idx]
    NC = max(1, (max(nv) + 127) // 128)
    NK = NC * 128

    kts = []
    vas = []
    for bb in range(B):
        kpad = np.zeros((NK, UNITS), np.float32)
        kpad[: nv[bb]] = memory[bb, :, :UNITS][idx[bb]]
        vpad = np.zeros((NK, UNITS), np.float32)
        vpad[: nv[bb]] = memory[bb, :, UNITS:][idx[bb]]
        ktr = kpad.T.reshape(H, DH, NK).transpose(1, 0, 2)  # [16, H, NK]
        aug = np.full((1, H, NK), NEG, np.float32)
        aug[:, :, : nv[bb]] = 0.0
        kts.append(np.ascontiguousarray(np.concatenate([ktr, aug], axis=0)))
        va_arr = np.zeros((NC, 128, H, VW), np.float32)
        va_arr[..., :DH] = vpad.reshape(NC, 128, H, DH)
        va_arr[..., 32] = 1.0
        vas.append(np.ascontiguousarray(va_arr.reshape(NC, 128, H * VW).astype(BF16)))

    in_maps = []
    for core in range(8):
        bb, qslot = divmod(core, 4)
        q0 = qslot * QPC
        qc = query[bb, q0 : q0 + QPC, :] * (DH ** -0.5)  # [1024, 128]
        qtr = qc.T.reshape(H, DH, QPC).transpose(1, 0, 2)  # [16, H, 1024]
        ones = np.ones((1, H, QPC), np.float32)
        qt_arr = np.ascontiguousarray(np.concatenate([qtr, ones], axis=0))
        in_maps.append({"kt": kts[bb], "qt": qt_arr, "va": vas[bb]})

    nc = _get_compiled(NC)
    from concourse.bass_utils import run_bass_kernel_spmd

    res = run_bass_kernel_spmd(
        nc, in_maps, core_ids=list(range(8)), trace=TRACE, tmpdir=TMPDIR
    )
    LAST = res

    out_full = np.empty((B, S, H * DH), np.float32)
    for core in range(8):
        bb, qslot = divmod(core, 4)
        o = res.results[core]["out"]  # [H, QPC//QT, DH, QT]
        o = o.transpose(1, 3, 0, 2).reshape(QPC, H * DH)
        out_full[bb, qslot * QPC : (qslot + 1) * QPC] = o
    return out_full
